# revision 1
# baseline (speedup 1.0000x reference)
"""GNN message-passing kernel for Trainium2 (8 NeuronCores, SPMD).

Computation (see reference):
  h1 = tanh(segsum(x[src] -> dst) @ W1 + b1)        [uses A(xW) = (Ax)W]
  h2 = tanh(segsum(h1[src] -> dst) @ W2 + b2)
  ht = logmap0(proj(h2))  (rowwise scale)
  pooled = segment mean over seg_ids, then expmap0/proj (host epilogue)

Sharding: nodes split contiguously over cores (dst-shard). Each core owns
SHARD nodes, processes the edges whose dst is in its shard.  The spmm is a
one-hot matmul: for each 128-edge tile, S^T[e,slot] = (dstslot[e]==slot)
(DVE is_equal vs iota), stationary lhsT=S^T, moving rhs = gathered rows.
Gather via gpsimd.dma_gather with int16 indices (tables chunked to 32768
rows).  The only cross-core exchange is one AllGather of h1 (bf16).
"""

import math
from contextlib import ExitStack

import numpy as np
import ml_dtypes

import concourse.bass as bass
import concourse.tile as tile
import concourse.bacc as bacc
from concourse import mybir

BF16 = mybir.dt.bfloat16
F32 = mybir.dt.float32
I16 = mybir.dt.int16
AF = mybir.ActivationFunctionType
ALU = mybir.AluOpType

MAXNORM = 1.0 - 1e-5
MIN_SS = 1e-15

SUB = 1024          # gather indices per dma_gather call (descriptor ring limit)
GRP = 4             # dst blocks (of 128 nodes) per PSUM group


class Cfg:
    def __init__(self, n_nodes, in_dim, hid, n_seg, n_cores):
        self.N = n_nodes
        self.IN = in_dim
        self.HID = hid
        self.NSEG = n_seg
        self.NC = n_cores
        self.SHARD = n_nodes // n_cores
        assert self.SHARD % 128 == 0
        self.NBLK = self.SHARD // 128
        assert self.NBLK % GRP == 0
        self.NGRP = self.NBLK // GRP
        self.CH = min(32768, n_nodes)
        assert n_nodes % self.CH == 0
        self.NCHUNK = n_nodes // self.CH
        self.NSEGCH = (n_seg + 127) // 128


def host_prep(cfg, src, dst):
    """Build SPMD-uniform edge tiling + per-core index/slot arrays.

    Returns (ntiles[NGRP,NCHUNK,GRP], per-core list of dicts with
    idx16 [128, TOT/16] int16 and dstslot [128, NTILES] float arrays).
    """
    NC, SHARD, CH = cfg.NC, cfg.SHARD, cfg.CH
    src = np.asarray(src).astype(np.int64)
    dst = np.asarray(dst).astype(np.int64)

    core = dst // SHARD
    blk = (dst % SHARD) // 128          # block within core [0, NBLK)
    slot = dst % 128
    chunk = src // CH
    idx = src % CH

    # counts[c, g, k, b]
    counts = np.zeros((NC, cfg.NGRP, cfg.NCHUNK, GRP), dtype=np.int64)
    g_all = blk // GRP
    b_all = blk % GRP
    np.add.at(counts, (core, g_all, chunk, b_all), 1)

    mx = counts.max(axis=0)
    ntiles = (mx + 127) // 128
    # ensure every block has >= 1 tile in chunk 0 (so PSUM gets a start write)
    empty = ntiles.sum(axis=1) == 0      # [NGRP, GRP]
    ntiles[:, 0, :][empty] = 1

    NTILES = int(ntiles.sum())
    TOT = NTILES * 128

    per_core = []
    # canonical ordering: g, k, b, then edges of that cell (+pad)
    order = np.lexsort((idx, b_all, chunk, g_all, core))
    # cell boundaries per core
    for c in range(NC):
        idx16 = np.zeros(TOT, dtype=np.int16)
        slots = np.full(TOT, -1.0, dtype=np.float32)
        sel = order[core[order] == c]
        csrc_idx = idx[sel]
        cslot = slot[sel]
        cg = g_all[sel]
        ck = chunk[sel]
        cb = b_all[sel]
        # counts per cell for this core
        ccnt = counts[c]
        pos = 0      # position in canonical padded stream
        ep = 0       # position in sel
        for g in range(cfg.NGRP):
            for k in range(cfg.NCHUNK):
                for b in range(GRP):
                    n = int(ccnt[g, k, b])
                    cap = int(ntiles[g, k, b]) * 128
                    if n > 0:
                        idx16[pos:pos + n] = csrc_idx[ep:ep + n]
                        slots[pos:pos + n] = cslot[ep:ep + n]
                        # sanity
                        assert np.all(cg[ep:ep + n] == g)
                        assert np.all(ck[ep:ep + n] == k)
                        assert np.all(cb[ep:ep + n] == b)
                        ep += n
                    pos += cap
        assert ep == len(sel)
        # wrap idx: i -> [i%16, i//16], replicate x8 partitions
        iw = idx16.reshape(-1, 16).T            # [16, TOT/16]
        iw = np.tile(iw, (8, 1)).copy()         # [128, TOT/16]
        # dstslot tile-major: [128 (edge in tile), NTILES]
        sl = slots.reshape(NTILES, 128).T.copy()
        per_core.append({"idx16": iw.astype(np.int16),
                         "dstslot": sl.astype(np.float32)})
    return ntiles, per_core


def _mm_schedule(cfg, ntiles):
    """Per (g): list over chunks of list of (tile_global_col, block b, start, stop)."""
    sched = []
    tcol = 0
    for g in range(cfg.NGRP):
        # first/last tile of each block across chunks
        tot_b = ntiles[g].sum(axis=0)   # [GRP]
        seen_b = np.zeros(GRP, dtype=np.int64)
        chunks = []
        for k in range(cfg.NCHUNK):
            tiles_k = []
            for b in range(GRP):
                for _ in range(int(ntiles[g, k, b])):
                    start = seen_b[b] == 0
                    stop = seen_b[b] == tot_b[b] - 1
                    tiles_k.append((tcol, b, bool(start), bool(stop)))
                    seen_b[b] += 1
                    tcol += 1
            chunks.append(tiles_k)
        sched.append(chunks)
    return sched


def build(cfg, ntiles, n_reps=1, debug_taps=False):
    """Build the Bass program. Returns nc."""
    N, IN, HID = cfg.N, cfg.IN, cfg.HID
    NTILES = int(ntiles.sum())
    TOT = NTILES * 128
    sched = _mm_schedule(cfg, ntiles)

    nc = bacc.Bacc("TRN2", target_bir_lowering=False)

    x_d = nc.dram_tensor("x_bf16", [N, IN], BF16, kind="ExternalInput")
    idx_d = nc.dram_tensor("idx16", [128, TOT // 16], I16, kind="ExternalInput")
    slot_d = nc.dram_tensor("dstslot", [128, NTILES], F32, kind="ExternalInput")
    segid_d = nc.dram_tensor("segid", [128, cfg.NBLK], F32, kind="ExternalInput")
    iota_d = nc.dram_tensor("iota128", [128, 128], BF16, kind="ExternalInput")
    iotas_d = nc.dram_tensor("iota_seg", [128, cfg.NSEGCH * 128], F32, kind="ExternalInput")
    ident_d = nc.dram_tensor("ident", [128, 128], BF16, kind="ExternalInput")
    w1_d = nc.dram_tensor("W1", [IN, HID], BF16, kind="ExternalInput")
    w2_d = nc.dram_tensor("W2", [HID, HID], BF16, kind="ExternalInput")
    b1_d = nc.dram_tensor("b1rep", [128, HID], F32, kind="ExternalInput")
    b2_d = nc.dram_tensor("b2rep", [128, HID], F32, kind="ExternalInput")

    h1_shard = nc.dram_tensor("h1_shard", [cfg.SHARD, HID], BF16)
    h1_full = nc.dram_tensor("h1_full", [N, HID], BF16, addr_space="Shared")
    out_d = nc.dram_tensor("pooled", [cfg.NSEGCH * 128, HID + 1], F32,
                           kind="ExternalOutput")
    if debug_taps:
        dbg_h1 = nc.dram_tensor("dbg_h1", [cfg.SHARD, HID], F32, kind="ExternalOutput")
        dbg_h2 = nc.dram_tensor("dbg_h2", [128, cfg.NBLK * HID], F32, kind="ExternalOutput")
        dbg_sc = nc.dram_tensor("dbg_sc", [128, 2 * cfg.NBLK], F32, kind="ExternalOutput")

    KIN = IN // 128   # k-chunks for W1 (2)

    with tile.TileContext(nc) as tc, ExitStack() as ctx:
        const = ctx.enter_context(tc.tile_pool(name="const", bufs=1))
        idxp = ctx.enter_context(tc.tile_pool(name="idxp", bufs=4))
        slotp = ctx.enter_context(tc.tile_pool(name="slotp", bufs=3))
        ebufp = ctx.enter_context(tc.tile_pool(name="ebufp", bufs=4))
        sp = ctx.enter_context(tc.tile_pool(name="sp", bufs=4))
        flshp = ctx.enter_context(tc.tile_pool(name="flshp", bufs=3))
        xtp = ctx.enter_context(tc.tile_pool(name="xtp", bufs=4))
        hp = ctx.enter_context(tc.tile_pool(name="hp", bufs=3))
        h2allp = ctx.enter_context(tc.tile_pool(name="h2allp", bufs=1))
        normp = ctx.enter_context(tc.tile_pool(name="normp", bufs=1))
        htp = ctx.enter_context(tc.tile_pool(name="htp", bufs=3))

        ctx_spmm = ctx.enter_context(ExitStack())
        ps_acc = ctx_spmm.enter_context(tc.tile_pool(name="ps_acc", bufs=4, space="PSUM"))
        ps_tr = ctx_spmm.enter_context(tc.tile_pool(name="ps_tr", bufs=1, space="PSUM"))
        ps_h = ctx_spmm.enter_context(tc.tile_pool(name="ps_h", bufs=2, space="PSUM"))

        # ---- constants ----
        iota128 = const.tile([128, 128], BF16)
        nc.sync.dma_start(iota128[:], iota_d[:])
        iotaseg = const.tile([128, cfg.NSEGCH * 128], F32)
        nc.sync.dma_start(iotaseg[:], iotas_d[:])
        ident = const.tile([128, 128], BF16)
        nc.sync.dma_start(ident[:], ident_d[:])
        segid = const.tile([128, cfg.NBLK], F32)
        nc.sync.dma_start(segid[:], segid_d[:])
        w1_sb = [const.tile([128, HID], BF16, tag=f"w1_{k}", name=f"w1_{k}")
                 for k in range(KIN)]
        for k in range(KIN):
            nc.sync.dma_start(w1_sb[k][:], w1_d[k * 128:(k + 1) * 128, :])
        w2_sb = const.tile([128, HID], BF16)
        nc.sync.dma_start(w2_sb[:], w2_d[:])
        b1_sb = const.tile([128, HID], F32)
        nc.sync.dma_start(b1_sb[:], b1_d[:])
        b2_sb = const.tile([128, HID], F32)
        nc.sync.dma_start(b2_sb[:], b2_d[:])

        h2_all = h2allp.tile([128, cfg.NBLK * HID], F32)
        norms2 = normp.tile([128, cfg.NBLK], F32)
        scale = normp.tile([128, cfg.NBLK], F32)
        na = normp.tile([128, cfg.NBLK], F32)
        nb_t = normp.tile([128, cfg.NBLK], F32)

        def spmm_layer(layer, table_ap, feat, out_block):
            """One spmm layer.  table_ap: DRAM [N, feat] gather table.
            out_block(g, b, agg_ps) consumes the accumulated [128(slot),
            feat] PSUM tile for global block nb=g*GRP+b.  One PSUM bank
            per block: start=True clears has_written bank-wide on HW, so
            accumulation groups must not share a bank."""
            for g in range(cfg.NGRP):
                accs = [ps_acc.tile([128, IN], F32, tag="acc", name=f"acc{b}")
                        for b in range(GRP)]

                def acc_slice(b):
                    return accs[b][:, :feat]

                for k in range(cfg.NCHUNK):
                    tiles_k = sched[g][k]
                    if not tiles_k:
                        continue
                    tbl = table_ap[k * cfg.CH:(k + 1) * cfg.CH, :]
                    # subcalls of <= SUB indices
                    for s0 in range(0, len(tiles_k), SUB // 128):
                        stiles = tiles_k[s0:s0 + SUB // 128]
                        nidx = len(stiles) * 128
                        col0 = stiles[0][0]  # global tile col
                        it = idxp.tile([128, SUB // 16], I16, tag="it")
                        nc.sync.dma_start(
                            it[:, :nidx // 16],
                            idx_d[:, col0 * 8:col0 * 8 + nidx // 16])
                        st = slotp.tile([128, SUB // 128], F32, tag="st")
                        nc.sync.dma_start(
                            st[:, :len(stiles)],
                            slot_d[:, col0:col0 + len(stiles)])
                        eb = ebufp.tile([128, (SUB // 128) * feat], BF16,
                                        tag=f"eb{layer}")
                        nc.gpsimd.dma_gather(
                            out_ap=eb[:, :len(stiles) * feat].rearrange(
                                "p (n f) -> p n f", f=feat),
                            in_ap=tbl,
                            idxs_ap=it[:, :nidx // 16],
                            num_idxs=nidx,
                            num_idxs_reg=nidx,
                            elem_size=feat,
                        )
                        for j, (tcol, b, st_f, sp_f) in enumerate(stiles):
                            s_t = sp.tile([128, 128], BF16, tag="s_t")
                            nc.vector.tensor_scalar(
                                s_t[:], iota128[:], st[:, j:j + 1], None,
                                ALU.is_equal)
                            nc.tensor.matmul(
                                acc_slice(b),
                                s_t[:],
                                eb[:, j * feat:(j + 1) * feat],
                                start=st_f, stop=sp_f,
                            )
                for b in range(GRP):
                    out_block(g, b, acc_slice(b))

        def l1_block(g, b, agg_ps):
            nb = g * GRP + b
            # copy PSUM f32 -> SBUF bf16
            ax = flshp.tile([128, IN], BF16, tag="ax1")
            nc.scalar.activation(ax[:], agg_ps, AF.Copy)
            h_ps = ps_h.tile([128, HID], F32, tag="hps", name="h_ps")
            for h in range(KIN):
                t_ps = ps_tr.tile([128, 128], BF16, tag="tps")
                nc.tensor.transpose(t_ps[:], ax[:, h * 128:(h + 1) * 128], ident[:])
                xt = xtp.tile([128, 128], BF16, tag="xt")
                nc.scalar.activation(xt[:], t_ps[:], AF.Copy)
                nc.tensor.matmul(h_ps[:], xt[:], w1_sb[h][:],
                                 start=(h == 0), stop=(h == KIN - 1))
            htmp = hp.tile([128, HID], F32, tag="htmp")
            nc.vector.tensor_add(htmp[:], h_ps[:], b1_sb[:])
            h1b = hp.tile([128, HID], BF16, tag="h1b")
            nc.scalar.activation(h1b[:], htmp[:], AF.Tanh)
            nc.sync.dma_start(h1_shard[nb * 128:(nb + 1) * 128, :], h1b[:])
            if debug_taps:
                h1f = hp.tile([128, HID], F32, tag="h1f")
                nc.scalar.activation(h1f[:], htmp[:], AF.Tanh)
                nc.sync.dma_start(dbg_h1[nb * 128:(nb + 1) * 128, :], h1f[:])

        def l2_block(g, b, agg_ps):
            nb = g * GRP + b
            a2 = flshp.tile([128, HID], BF16, tag="a22")
            nc.scalar.activation(a2[:], agg_ps, AF.Copy)
            t_ps = ps_tr.tile([128, 128], BF16, tag="tps")
            nc.tensor.transpose(t_ps[:], a2[:], ident[:])
            a2t = xtp.tile([128, 128], BF16, tag="xt")
            nc.scalar.activation(a2t[:], t_ps[:], AF.Copy)
            h_ps = ps_h.tile([128, HID], F32, tag="hps", name="h_ps")
            nc.tensor.matmul(h_ps[:], a2t[:], w2_sb[:], start=True, stop=True)
            htmp = hp.tile([128, HID], F32, tag="htmp")
            nc.vector.tensor_add(htmp[:], h_ps[:], b2_sb[:])
            nc.scalar.activation(h2_all[:, nb * HID:(nb + 1) * HID], htmp[:],
                                 AF.Tanh)

        # ---------------- layer 1 ----------------
        spmm_layer(1, x_d, IN, l1_block)

        # ---------------- exchange ----------------
        nc.gpsimd.collective_compute(
            "AllGather",
            ALU.bypass,
            ins=[h1_shard.ap().opt()],
            outs=[h1_full.ap().opt()],
            replica_groups=[list(range(cfg.NC))],
        )

        # ---------------- layer 2 ----------------
        spmm_layer(2, h1_full, HID, l2_block)

        # ---------------- norms + logmap scale ----------------
        for nbk in range(cfg.NBLK):
            h2b = h2_all[:, nbk * HID:(nbk + 1) * HID]
            sq = htp.tile([128, HID], F32, tag="sq")
            nc.vector.tensor_mul(sq[:], h2b, h2b)
            nc.vector.tensor_reduce(norms2[:, nbk:nbk + 1], sq[:],
                                    mybir.AxisListType.X, ALU.add)
        # norm = sqrt(max(ss, MIN_SS)); nclip = min(norm, MAXNORM)
        nc.vector.tensor_scalar_max(na[:], norms2[:], MIN_SS)
        nc.scalar.activation(nb_t[:], na[:], AF.Sqrt)        # nb_t = norm
        nc.vector.tensor_scalar_min(na[:], nb_t[:], MAXNORM)  # na = nclip
        # artanh(nclip) = 0.5*ln((1+n)/(1-n)); scale = artanh/norm
        one_m = normp.tile([128, cfg.NBLK], F32)
        nc.vector.tensor_scalar(one_m[:], na[:], -1.0, 1.0, ALU.mult, ALU.add)
        one_p = normp.tile([128, cfg.NBLK], F32)
        nc.vector.tensor_scalar_add(one_p[:], na[:], 1.0)
        rcp = normp.tile([128, cfg.NBLK], F32)
        nc.vector.reciprocal(rcp[:], one_m[:])
        rat = normp.tile([128, cfg.NBLK], F32)
        nc.vector.tensor_mul(rat[:], one_p[:], rcp[:])
        lg = normp.tile([128, cfg.NBLK], F32)
        nc.scalar.activation(lg[:], rat[:], AF.Ln)
        nc.vector.tensor_scalar_mul(lg[:], lg[:], 0.5)
        rcpn = normp.tile([128, cfg.NBLK], F32)
        nc.vector.reciprocal(rcpn[:], nb_t[:])
        nc.vector.tensor_mul(scale[:], lg[:], rcpn[:])

        if debug_taps:
            nc.sync.dma_start(dbg_h2[:], h2_all[:])
            nc.sync.dma_start(dbg_sc[:, :cfg.NBLK], norms2[:])
            nc.sync.dma_start(dbg_sc[:, cfg.NBLK:], scale[:])
        # ---------------- pooling ----------------
        ctx_spmm.close()
        ps_pool = ctx.enter_context(
            tc.tile_pool(name="ps_pool", bufs=max(cfg.NSEGCH, 1), space="PSUM"))
        pool_ps = [ps_pool.tile([128, HID + 1], F32, tag="pool", name=f"pool{sc}")
                   for sc in range(cfg.NSEGCH)]
        for nbk in range(cfg.NBLK):
            h2b = h2_all[:, nbk * HID:(nbk + 1) * HID]
            ht = htp.tile([128, HID + 1], BF16, tag="ht")
            nc.vector.tensor_scalar(ht[:, :HID], h2b, scale[:, nbk:nbk + 1],
                                    None, ALU.mult)
            nc.vector.memset(ht[:, HID:HID + 1], 1.0)
            for sc in range(cfg.NSEGCH):
                sg = sp.tile([128, 128], BF16, tag="sg")
                nc.vector.tensor_scalar(
                    sg[:], iotaseg[:, sc * 128:(sc + 1) * 128],
                    segid[:, nbk:nbk + 1], None, ALU.is_equal)
                nc.tensor.matmul(
                    pool_ps[sc][:], sg[:], ht[:],
                    start=(nbk == 0), stop=(nbk == cfg.NBLK - 1))
        for sc in range(cfg.NSEGCH):
            po = htp.tile([128, HID + 1], F32, tag="po")
            nc.vector.tensor_copy(po[:], pool_ps[sc][:])
            nc.sync.dma_start(out_d[sc * 128:(sc + 1) * 128, :], po[:])

    nc.compile()
    return nc


def host_inputs(cfg, x, seg_ids, W1, b1, W2, b2, per_core):
    """Per-core in_maps for run_bass_kernel_spmd."""
    N, IN, HID = cfg.N, cfg.IN, cfg.HID
    x_bf16 = np.ascontiguousarray(x.astype(ml_dtypes.bfloat16))
    iota128 = np.tile(np.arange(128, dtype=np.float32), (128, 1)).astype(ml_dtypes.bfloat16)
    iotaseg = np.tile(np.arange(cfg.NSEGCH * 128, dtype=np.float32), (128, 1))
    ident = np.eye(128, dtype=np.float32).astype(ml_dtypes.bfloat16)
    w1 = np.ascontiguousarray(W1.astype(ml_dtypes.bfloat16))
    w2 = np.ascontiguousarray(W2.astype(ml_dtypes.bfloat16))
    b1r = np.tile(np.asarray(b1, np.float32), (128, 1))
    b2r = np.tile(np.asarray(b2, np.float32), (128, 1))
    seg = np.asarray(seg_ids, np.float32)
    maps = []
    for c in range(cfg.NC):
        segc = seg[c * cfg.SHARD:(c + 1) * cfg.SHARD].reshape(cfg.NBLK, 128).T
        maps.append({
            "x_bf16": x_bf16,
            "idx16": per_core[c]["idx16"],
            "dstslot": per_core[c]["dstslot"],
            "segid": np.ascontiguousarray(segc),
            "iota128": iota128,
            "iota_seg": np.ascontiguousarray(iotaseg.astype(np.float32)),
            "ident": ident,
            "W1": w1,
            "W2": w2,
            "b1rep": b1r,
            "b2rep": b2r,
        })
    return maps


def host_epilogue(cfg, partials, batch_size, max_comments):
    """partials: list of per-core [NSEGCH*128, HID+1] f32."""
    acc = np.zeros_like(partials[0], dtype=np.float64)
    for p in partials:
        acc += p.astype(np.float64)
    acc = acc.astype(np.float32)
    nseg = cfg.NSEG
    sums = acc[:nseg, :cfg.HID]
    counts = acc[:nseg, cfg.HID]
    agg = sums / np.maximum(counts, 1.0)[:, None]
    # expmap0 then proj
    ss = np.maximum(np.sum(agg * agg, axis=1), MIN_SS).astype(np.float32)
    norm = np.sqrt(ss)
    y = agg * (np.tanh(norm) / norm)[:, None]
    ssy = np.maximum(np.sum(y * y, axis=1), MIN_SS).astype(np.float32)
    ny = np.sqrt(ssy)
    f = np.where(ny > MAXNORM, MAXNORM / ny, 1.0).astype(np.float32)
    y = y * f[:, None]
    return y.reshape(int(batch_size), int(max_comments), cfg.HID)


# ---------------- numpy reference (for arbitrary sizes) ----------------

def np_reference(x, src, dst, seg_ids, W1, b1, W2, b2, batch_size, max_comments):
    n = x.shape[0]

    def seg_sum(vals, ids, nseg):
        out = np.zeros((nseg, vals.shape[1]), np.float32)
        np.add.at(out, ids, vals)
        return out

    def rownorm(v):
        return np.sqrt(np.maximum(np.sum(v * v, axis=1, keepdims=True), MIN_SS))

    def proj(v):
        nn = rownorm(v)
        return np.where(nn > MAXNORM, v / nn * MAXNORM, v)

    def logmap0(v):
        nn = rownorm(v)
        arg = np.minimum(nn, 1 - 1e-7)
        return v * np.arctanh(arg) / nn

    def expmap0(v):
        nn = rownorm(v)
        return v * np.tanh(nn) / nn

    h = np.tanh(seg_sum(x[src] @ W1, dst, n) + b1)
    h = np.tanh(seg_sum(h[src] @ W2, dst, n) + b2)
    h = logmap0(proj(h))
    nseg = int(batch_size) * int(max_comments)
    sums = seg_sum(h, seg_ids, nseg)
    counts = np.zeros(nseg, np.float32)
    np.add.at(counts, seg_ids, 1.0)
    agg = sums / np.maximum(counts, 1.0)[:, None]
    agg = proj(expmap0(agg))
    return agg.reshape(int(batch_size), int(max_comments), -1)


# ====================================================================
# Harness entry point: kernel(**inputs) -> np.ndarray
# ====================================================================

_CACHE = {}


def kernel(x, src, dst, seg_ids, W1, b1, W2, b2, batch_size, max_comments):
    """Full-input GNN ComEnc kernel on 8 Trainium2 NeuronCores.

    Accepts the unsharded inputs of reference.setup_inputs() and returns
    the full (batch, max_comments, HID) float32 output.
    """
    from concourse.bass_utils import run_bass_kernel_spmd

    x = np.asarray(x, dtype=np.float32)
    src = np.asarray(src).astype(np.int64)
    dst = np.asarray(dst).astype(np.int64)
    seg_ids = np.asarray(seg_ids).astype(np.int64)
    W1 = np.asarray(W1, dtype=np.float32)
    b1 = np.asarray(b1, dtype=np.float32)
    W2 = np.asarray(W2, dtype=np.float32)
    b2 = np.asarray(b2, dtype=np.float32)
    bs = int(np.asarray(batch_size))
    mc = int(np.asarray(max_comments))

    n_nodes, in_dim = x.shape
    hid = W1.shape[1]
    nseg = bs * mc
    n_cores = 8

    cfg = Cfg(n_nodes, in_dim, hid, nseg, n_cores)
    ntiles, per_core = host_prep(cfg, src, dst)

    key = (n_nodes, in_dim, hid, nseg, ntiles.tobytes())
    if key in _CACHE:
        nc = _CACHE[key]
    else:
        nc = build(cfg, ntiles)
        _CACHE.clear()
        _CACHE[key] = nc

    maps = host_inputs(cfg, x, seg_ids, W1, b1, W2, b2, per_core)
    res = run_bass_kernel_spmd(nc, maps, core_ids=list(range(n_cores)))
    partials = [r["pooled"] for r in res.results]
    out = host_epilogue(cfg, partials, bs, mc)
    return np.ascontiguousarray(out.astype(np.float32))



# revision 11
# speedup vs baseline: 1.0495x; 1.0495x over previous
"""GNN message-passing kernel for Trainium2 (8 NeuronCores, SPMD).

Computation (see reference):
  h1 = tanh(segsum(x[src] -> dst) @ W1 + b1)        [uses A(xW) = (Ax)W]
  h2 = tanh(segsum(h1[src] -> dst) @ W2 + b2)
  ht = logmap0(proj(h2))  (rowwise scale)
  pooled = segment mean over seg_ids, then expmap0/proj (host epilogue)

Sharding: nodes split contiguously over cores (dst-shard). Each core owns
SHARD nodes, processes the edges whose dst is in its shard.  The spmm is a
one-hot matmul: for each 128-edge tile, S^T[e,slot] = (dstslot[e]==slot)
(DVE is_equal vs iota), stationary lhsT=S^T, moving rhs = gathered rows.
Gather via gpsimd.dma_gather with int16 indices (tables chunked to 32768
rows).  The only cross-core exchange is one AllGather of h1 (bf16).
"""

import math
from contextlib import ExitStack

import numpy as np
import ml_dtypes

import concourse.bass as bass
import concourse.tile as tile
import concourse.bacc as bacc
from concourse import mybir

BF16 = mybir.dt.bfloat16
F32 = mybir.dt.float32
I16 = mybir.dt.int16
AF = mybir.ActivationFunctionType
ALU = mybir.AluOpType

MAXNORM = 1.0 - 1e-5
MIN_SS = 1e-15

SUB = 1024          # gather indices per dma_gather call (descriptor ring limit)
GRP = 4             # dst blocks (of 128 nodes) per PSUM group


class Cfg:
    def __init__(self, n_nodes, in_dim, hid, n_seg, n_cores):
        self.N = n_nodes
        self.IN = in_dim
        self.HID = hid
        self.NSEG = n_seg
        self.NC = n_cores
        self.SHARD = n_nodes // n_cores
        assert self.SHARD % 128 == 0
        self.NBLK = self.SHARD // 128
        assert self.NBLK % GRP == 0
        self.NGRP = self.NBLK // GRP
        self.CH = min(32768, n_nodes)
        assert n_nodes % self.CH == 0
        self.NCHUNK = n_nodes // self.CH
        self.NSEGCH = (n_seg + 127) // 128


def host_prep(cfg, src, dst):
    """Build SPMD-uniform edge tiling + per-core index/slot arrays.

    Returns (ntiles[NGRP,NCHUNK,GRP], per-core list of dicts with
    idx16 [128, TOT/16] int16 and dstslot [128, NTILES] float arrays).
    """
    NC, SHARD, CH = cfg.NC, cfg.SHARD, cfg.CH
    src = np.asarray(src).astype(np.int64)
    dst = np.asarray(dst).astype(np.int64)

    core = dst // SHARD
    blk = (dst % SHARD) // 128          # block within core [0, NBLK)
    slot = dst % 128
    chunk = src // CH
    idx = src % CH

    # counts[c, g, k, b]
    counts = np.zeros((NC, cfg.NGRP, cfg.NCHUNK, GRP), dtype=np.int64)
    g_all = blk // GRP
    b_all = blk % GRP
    np.add.at(counts, (core, g_all, chunk, b_all), 1)

    mx = counts.max(axis=0)
    ntiles = (mx + 127) // 128
    # ensure every block has >= 1 tile in chunk 0 (so PSUM gets a start write)
    empty = ntiles.sum(axis=1) == 0      # [NGRP, GRP]
    ntiles[:, 0, :][empty] = 1

    NTILES = int(ntiles.sum())
    TOT = NTILES * 128

    per_core = []
    # canonical ordering: g, k, b, then edges of that cell (+pad)
    order = np.lexsort((idx, b_all, chunk, g_all, core))
    # cell boundaries per core
    for c in range(NC):
        idx16 = np.zeros(TOT, dtype=np.int16)
        slots = np.full(TOT, -1.0, dtype=np.float32)
        sel = order[core[order] == c]
        csrc_idx = idx[sel]
        cslot = slot[sel]
        cg = g_all[sel]
        ck = chunk[sel]
        cb = b_all[sel]
        # counts per cell for this core
        ccnt = counts[c]
        pos = 0      # position in canonical padded stream
        ep = 0       # position in sel
        for g in range(cfg.NGRP):
            for k in range(cfg.NCHUNK):
                for b in range(GRP):
                    n = int(ccnt[g, k, b])
                    cap = int(ntiles[g, k, b]) * 128
                    if n > 0:
                        idx16[pos:pos + n] = csrc_idx[ep:ep + n]
                        slots[pos:pos + n] = cslot[ep:ep + n]
                        # sanity
                        assert np.all(cg[ep:ep + n] == g)
                        assert np.all(ck[ep:ep + n] == k)
                        assert np.all(cb[ep:ep + n] == b)
                        ep += n
                    pos += cap
        assert ep == len(sel)
        # wrap idx: i -> [i%16, i//16], replicate x8 partitions
        iw = idx16.reshape(-1, 16).T            # [16, TOT/16]
        iw = np.tile(iw, (8, 1)).copy()         # [128, TOT/16]
        # dstslot tile-major: [128 (edge in tile), NTILES]
        sl = slots.reshape(NTILES, 128).T.copy()
        per_core.append({"idx16": iw.astype(np.int16),
                         "dstslot": sl.astype(np.float32)})
    return ntiles, per_core


def _mm_schedule(cfg, ntiles):
    """Per (g): list over chunks of list of (tile_global_col, block b, start, stop)."""
    sched = []
    tcol = 0
    for g in range(cfg.NGRP):
        # first/last tile of each block across chunks
        tot_b = ntiles[g].sum(axis=0)   # [GRP]
        seen_b = np.zeros(GRP, dtype=np.int64)
        chunks = []
        for k in range(cfg.NCHUNK):
            tiles_k = []
            for b in range(GRP):
                for _ in range(int(ntiles[g, k, b])):
                    start = seen_b[b] == 0
                    stop = seen_b[b] == tot_b[b] - 1
                    tiles_k.append((tcol, b, bool(start), bool(stop)))
                    seen_b[b] += 1
                    tcol += 1
            chunks.append(tiles_k)
        sched.append(chunks)
    return sched


def build(cfg, ntiles, n_reps=1, debug_taps=False):
    """Build the Bass program. Returns nc."""
    N, IN, HID = cfg.N, cfg.IN, cfg.HID
    NTILES = int(ntiles.sum())
    TOT = NTILES * 128
    sched = _mm_schedule(cfg, ntiles)
    cfg.MAXGT = int(ntiles.sum(axis=(1, 2)).max())

    nc = bacc.Bacc("TRN2", target_bir_lowering=False,
                   dynamic_dma_scratch_size=SUB * 16)

    x_d = nc.dram_tensor("x_bf16", [N, IN], BF16, kind="ExternalInput")
    idx_d = nc.dram_tensor("idx16", [128, TOT // 16], I16, kind="ExternalInput")
    slot_d = nc.dram_tensor("dstslot", [128, NTILES], F32, kind="ExternalInput")
    segid_d = nc.dram_tensor("segid", [128, cfg.NBLK], F32, kind="ExternalInput")
    iota_d = nc.dram_tensor("iota128", [128, 128], BF16, kind="ExternalInput")
    iotas_d = nc.dram_tensor("iota_seg", [128, cfg.NSEGCH * 128], F32, kind="ExternalInput")
    ident_d = nc.dram_tensor("ident", [128, 128], BF16, kind="ExternalInput")
    w1_d = nc.dram_tensor("W1", [IN, HID], BF16, kind="ExternalInput")
    w2_d = nc.dram_tensor("W2", [HID, HID], BF16, kind="ExternalInput")
    b1_d = nc.dram_tensor("b1rep", [128, HID], F32, kind="ExternalInput")
    b2_d = nc.dram_tensor("b2rep", [128, HID], F32, kind="ExternalInput")

    h1_shard = nc.dram_tensor("h1_shard", [cfg.SHARD, HID], BF16)
    h1_full = nc.dram_tensor("h1_full", [N, HID], BF16, addr_space="Shared")
    out_d = nc.dram_tensor("pooled", [cfg.NSEGCH * 128, HID + 1], F32,
                           kind="ExternalOutput")
    if debug_taps:
        dbg_h1 = nc.dram_tensor("dbg_h1", [cfg.SHARD, HID], F32, kind="ExternalOutput")
        dbg_h2 = nc.dram_tensor("dbg_h2", [128, cfg.NBLK * HID], F32, kind="ExternalOutput")
        dbg_sc = nc.dram_tensor("dbg_sc", [128, 2 * cfg.NBLK], F32, kind="ExternalOutput")

    KIN = IN // 128   # k-chunks for W1 (2)

    with tile.TileContext(nc) as tc, ExitStack() as ctx:
        const = ctx.enter_context(tc.tile_pool(name="const", bufs=1))
        idxp = ctx.enter_context(tc.tile_pool(name="idxp", bufs=4))
        slotp = ctx.enter_context(tc.tile_pool(name="slotp", bufs=3))
        ebufp = ctx.enter_context(tc.tile_pool(name="ebufp", bufs=3))
        sp = ctx.enter_context(tc.tile_pool(name="sp", bufs=4))
        flshp = ctx.enter_context(tc.tile_pool(name="flshp", bufs=3))
        xtp = ctx.enter_context(tc.tile_pool(name="xtp", bufs=4))
        hp = ctx.enter_context(tc.tile_pool(name="hp", bufs=3))
        h2allp = ctx.enter_context(tc.tile_pool(name="h2allp", bufs=1))
        normp = ctx.enter_context(tc.tile_pool(name="normp", bufs=1))
        htp = ctx.enter_context(tc.tile_pool(name="htp", bufs=3))

        ctx_spmm = ctx.enter_context(ExitStack())
        ps_acc = ctx_spmm.enter_context(tc.tile_pool(name="ps_acc", bufs=4, space="PSUM"))
        ps_tr = ctx_spmm.enter_context(tc.tile_pool(name="ps_tr", bufs=2, space="PSUM"))
        ps_h = ctx_spmm.enter_context(tc.tile_pool(name="ps_h", bufs=2, space="PSUM"))

        # ---- constants ----
        iota128 = const.tile([128, 128], BF16)
        nc.sync.dma_start(iota128[:], iota_d[:])
        iotaseg = const.tile([128, cfg.NSEGCH * 128], F32)
        nc.sync.dma_start(iotaseg[:], iotas_d[:])
        ident = const.tile([128, 128], BF16)
        nc.sync.dma_start(ident[:], ident_d[:])
        segid = const.tile([128, cfg.NBLK], F32)
        nc.sync.dma_start(segid[:], segid_d[:])
        w1_sb = [const.tile([128, HID], BF16, tag=f"w1_{k}", name=f"w1_{k}")
                 for k in range(KIN)]
        for k in range(KIN):
            nc.sync.dma_start(w1_sb[k][:], w1_d[k * 128:(k + 1) * 128, :])
        w2_sb = const.tile([128, HID], BF16)
        nc.sync.dma_start(w2_sb[:], w2_d[:])
        b1_sb = const.tile([128, HID], F32)
        nc.sync.dma_start(b1_sb[:], b1_d[:])
        b2_sb = const.tile([128, HID], F32)
        nc.sync.dma_start(b2_sb[:], b2_d[:])

        h2_all = h2allp.tile([128, cfg.NBLK * HID], BF16)
        norms2 = normp.tile([128, cfg.NBLK], F32)
        scale = normp.tile([128, cfg.NBLK], F32)
        na = normp.tile([128, cfg.NBLK], F32)
        nb_t = normp.tile([128, cfg.NBLK], F32)

        def spmm_layer(layer, table_ap, feat, out_block):
            """One spmm layer.  table_ap: DRAM [N, feat] gather table.
            out_block(g, b, agg_ps) consumes the accumulated [128(slot),
            feat] PSUM tile for global block nb=g*GRP+b.  One PSUM bank
            per block: start=True clears has_written bank-wide on HW, so
            accumulation groups must not share a bank."""
            for g in range(cfg.NGRP):
                accs = [ps_acc.tile([128, IN], F32, tag="acc", name=f"acc{b}")
                        for b in range(GRP)]

                def acc_slice(b):
                    return accs[b][:, :feat]

                # one idx + slot load covering the whole group
                gtiles = sum(len(sched[g][k]) for k in range(cfg.NCHUNK))
                gcol0 = None
                for k in range(cfg.NCHUNK):
                    if sched[g][k]:
                        gcol0 = sched[g][k][0][0]
                        break
                it_g = idxp.tile([128, (cfg.MAXGT + 31) * 8], I16, tag="it")
                nc.sync.dma_start(
                    it_g[:, :gtiles * 8],
                    idx_d[:, gcol0 * 8:(gcol0 + gtiles) * 8])
                st_g = slotp.tile([128, cfg.MAXGT + 31], F32, tag="st")
                nc.sync.dma_start(
                    st_g[:, :gtiles],
                    slot_d[:, gcol0:gcol0 + gtiles])

                for k in range(cfg.NCHUNK):
                    tiles_k = sched[g][k]
                    if not tiles_k:
                        continue
                    tbl = table_ap[k * cfg.CH:(k + 1) * cfg.CH, :]
                    # subcalls of <= SUB indices
                    for s0 in range(0, len(tiles_k), SUB // 128):
                        stiles = tiles_k[s0:s0 + SUB // 128]
                        nidx = len(stiles) * 128
                        lt0 = stiles[0][0] - gcol0  # tile idx within group
                        eb = ebufp.tile([128, (SUB // 128) * feat], BF16,
                                        tag=f"eb{layer}")
                        nc.gpsimd.dma_gather(
                            out_ap=eb[:, :len(stiles) * feat].rearrange(
                                "p (n f) -> p n f", f=feat),
                            in_ap=tbl,
                            idxs_ap=it_g[:, lt0 * 8:lt0 * 8 + nidx // 16],
                            num_idxs=nidx,
                            num_idxs_reg=nidx,
                            elem_size=feat,
                        )
                        for j, (tcol, b, st_f, sp_f) in enumerate(stiles):
                            lt = tcol - gcol0
                            s_t = sp.tile([128, 128], BF16, tag="s_t")
                            nc.vector.tensor_scalar(
                                s_t[:], iota128[:], st_g[:, lt:lt + 1], None,
                                ALU.is_equal)
                            nc.tensor.matmul(
                                acc_slice(b),
                                s_t[:],
                                eb[:, j * feat:(j + 1) * feat],
                                start=st_f, stop=sp_f,
                            )
                for b in range(GRP):
                    out_block(g, b, acc_slice(b))

        def l1_block(g, b, agg_ps):
            nb = g * GRP + b
            # copy PSUM f32 -> SBUF bf16
            ax = flshp.tile([128, IN], BF16, tag="ax1")
            nc.scalar.activation(ax[:], agg_ps, AF.Copy)
            h_ps = ps_h.tile([128, HID], F32, tag="hps", name="h_ps")
            for h in range(KIN):
                t_ps = ps_tr.tile([128, 128], BF16, tag="tps")
                nc.tensor.transpose(t_ps[:], ax[:, h * 128:(h + 1) * 128], ident[:])
                xt = xtp.tile([128, 128], BF16, tag="xt")
                nc.scalar.activation(xt[:], t_ps[:], AF.Copy)
                nc.tensor.matmul(h_ps[:], xt[:], w1_sb[h][:],
                                 start=(h == 0), stop=(h == KIN - 1))
            htmp = hp.tile([128, HID], F32, tag="htmp")
            nc.vector.tensor_add(htmp[:], h_ps[:], b1_sb[:])
            h1b = hp.tile([128, HID], BF16, tag="h1b")
            nc.scalar.activation(h1b[:], htmp[:], AF.Tanh)
            nc.sync.dma_start(h1_shard[nb * 128:(nb + 1) * 128, :], h1b[:])
            if debug_taps:
                h1f = hp.tile([128, HID], F32, tag="h1f")
                nc.scalar.activation(h1f[:], htmp[:], AF.Tanh)
                nc.sync.dma_start(dbg_h1[nb * 128:(nb + 1) * 128, :], h1f[:])

        def l2_block(g, b, agg_ps):
            nb = g * GRP + b
            a2 = flshp.tile([128, HID], BF16, tag="a22")
            nc.scalar.activation(a2[:], agg_ps, AF.Copy)
            t_ps = ps_tr.tile([128, 128], BF16, tag="tps")
            nc.tensor.transpose(t_ps[:], a2[:], ident[:])
            a2t = xtp.tile([128, 128], BF16, tag="xt")
            nc.scalar.activation(a2t[:], t_ps[:], AF.Copy)
            h_ps = ps_h.tile([128, HID], F32, tag="hps", name="h_ps")
            nc.tensor.matmul(h_ps[:], a2t[:], w2_sb[:], start=True, stop=True)
            htmp = hp.tile([128, HID], F32, tag="htmp")
            nc.vector.tensor_add(htmp[:], h_ps[:], b2_sb[:])
            nc.scalar.activation(h2_all[:, nb * HID:(nb + 1) * HID], htmp[:],
                                 AF.Tanh)

        # ---------------- layer 1 ----------------
        spmm_layer(1, x_d, IN, l1_block)

        # ---------------- exchange ----------------
        nc.gpsimd.collective_compute(
            "AllGather",
            ALU.bypass,
            ins=[h1_shard.ap().opt()],
            outs=[h1_full.ap().opt()],
            replica_groups=[list(range(cfg.NC))],
        )

        # ---------------- layer 2 ----------------
        spmm_layer(2, h1_full, HID, l2_block)

        # ---------------- norms + logmap scale ----------------
        for nbk in range(cfg.NBLK):
            h2b = h2_all[:, nbk * HID:(nbk + 1) * HID]
            sq = htp.tile([128, HID], F32, tag="sq")
            nc.vector.tensor_mul(sq[:], h2b, h2b)
            nc.vector.tensor_reduce(norms2[:, nbk:nbk + 1], sq[:],
                                    mybir.AxisListType.X, ALU.add)
        # norm = sqrt(max(ss, MIN_SS)); nclip = min(norm, MAXNORM)
        nc.vector.tensor_scalar_max(na[:], norms2[:], MIN_SS)
        nc.scalar.activation(nb_t[:], na[:], AF.Sqrt)        # nb_t = norm
        nc.vector.tensor_scalar_min(na[:], nb_t[:], MAXNORM)  # na = nclip
        # artanh(nclip) = 0.5*ln((1+n)/(1-n)); scale = artanh/norm
        one_m = normp.tile([128, cfg.NBLK], F32)
        nc.vector.tensor_scalar(one_m[:], na[:], -1.0, 1.0, ALU.mult, ALU.add)
        one_p = normp.tile([128, cfg.NBLK], F32)
        nc.vector.tensor_scalar_add(one_p[:], na[:], 1.0)
        rcp = normp.tile([128, cfg.NBLK], F32)
        nc.vector.reciprocal(rcp[:], one_m[:])
        rat = normp.tile([128, cfg.NBLK], F32)
        nc.vector.tensor_mul(rat[:], one_p[:], rcp[:])
        lg = normp.tile([128, cfg.NBLK], F32)
        nc.scalar.activation(lg[:], rat[:], AF.Ln)
        nc.vector.tensor_scalar_mul(lg[:], lg[:], 0.5)
        rcpn = normp.tile([128, cfg.NBLK], F32)
        nc.vector.reciprocal(rcpn[:], nb_t[:])
        nc.vector.tensor_mul(scale[:], lg[:], rcpn[:])

        if debug_taps:
            nc.sync.dma_start(dbg_h2[:], h2_all[:])
            nc.sync.dma_start(dbg_sc[:, :cfg.NBLK], norms2[:])
            nc.sync.dma_start(dbg_sc[:, cfg.NBLK:], scale[:])
        # ---------------- pooling ----------------
        ctx_spmm.close()
        ps_pool = ctx.enter_context(
            tc.tile_pool(name="ps_pool", bufs=max(cfg.NSEGCH, 1), space="PSUM"))
        pool_ps = [ps_pool.tile([128, HID + 1], F32, tag="pool", name=f"pool{sc}")
                   for sc in range(cfg.NSEGCH)]
        for nbk in range(cfg.NBLK):
            h2b = h2_all[:, nbk * HID:(nbk + 1) * HID]
            ht = htp.tile([128, HID + 1], BF16, tag="ht")
            nc.vector.tensor_scalar(ht[:, :HID], h2b, scale[:, nbk:nbk + 1],
                                    None, ALU.mult)
            nc.vector.memset(ht[:, HID:HID + 1], 1.0)
            for sc in range(cfg.NSEGCH):
                sg = sp.tile([128, 128], BF16, tag="sg")
                nc.vector.tensor_scalar(
                    sg[:], iotaseg[:, sc * 128:(sc + 1) * 128],
                    segid[:, nbk:nbk + 1], None, ALU.is_equal)
                nc.tensor.matmul(
                    pool_ps[sc][:], sg[:], ht[:],
                    start=(nbk == 0), stop=(nbk == cfg.NBLK - 1))
        for sc in range(cfg.NSEGCH):
            po = htp.tile([128, HID + 1], F32, tag="po")
            nc.vector.tensor_copy(po[:], pool_ps[sc][:])
            nc.sync.dma_start(out_d[sc * 128:(sc + 1) * 128, :], po[:])

    nc.compile()
    return nc


def host_inputs(cfg, x, seg_ids, W1, b1, W2, b2, per_core):
    """Per-core in_maps for run_bass_kernel_spmd."""
    N, IN, HID = cfg.N, cfg.IN, cfg.HID
    x_bf16 = np.ascontiguousarray(x.astype(ml_dtypes.bfloat16))
    iota128 = np.tile(np.arange(128, dtype=np.float32), (128, 1)).astype(ml_dtypes.bfloat16)
    iotaseg = np.tile(np.arange(cfg.NSEGCH * 128, dtype=np.float32), (128, 1))
    ident = np.eye(128, dtype=np.float32).astype(ml_dtypes.bfloat16)
    w1 = np.ascontiguousarray(W1.astype(ml_dtypes.bfloat16))
    w2 = np.ascontiguousarray(W2.astype(ml_dtypes.bfloat16))
    b1r = np.tile(np.asarray(b1, np.float32), (128, 1))
    b2r = np.tile(np.asarray(b2, np.float32), (128, 1))
    seg = np.asarray(seg_ids, np.float32)
    maps = []
    for c in range(cfg.NC):
        segc = seg[c * cfg.SHARD:(c + 1) * cfg.SHARD].reshape(cfg.NBLK, 128).T
        maps.append({
            "x_bf16": x_bf16,
            "idx16": per_core[c]["idx16"],
            "dstslot": per_core[c]["dstslot"],
            "segid": np.ascontiguousarray(segc),
            "iota128": iota128,
            "iota_seg": np.ascontiguousarray(iotaseg.astype(np.float32)),
            "ident": ident,
            "W1": w1,
            "W2": w2,
            "b1rep": b1r,
            "b2rep": b2r,
        })
    return maps


def host_epilogue(cfg, partials, batch_size, max_comments):
    """partials: list of per-core [NSEGCH*128, HID+1] f32."""
    acc = np.zeros_like(partials[0], dtype=np.float64)
    for p in partials:
        acc += p.astype(np.float64)
    acc = acc.astype(np.float32)
    nseg = cfg.NSEG
    sums = acc[:nseg, :cfg.HID]
    counts = acc[:nseg, cfg.HID]
    agg = sums / np.maximum(counts, 1.0)[:, None]
    # expmap0 then proj
    ss = np.maximum(np.sum(agg * agg, axis=1), MIN_SS).astype(np.float32)
    norm = np.sqrt(ss)
    y = agg * (np.tanh(norm) / norm)[:, None]
    ssy = np.maximum(np.sum(y * y, axis=1), MIN_SS).astype(np.float32)
    ny = np.sqrt(ssy)
    f = np.where(ny > MAXNORM, MAXNORM / ny, 1.0).astype(np.float32)
    y = y * f[:, None]
    return y.reshape(int(batch_size), int(max_comments), cfg.HID)


# ---------------- numpy reference (for arbitrary sizes) ----------------

def np_reference(x, src, dst, seg_ids, W1, b1, W2, b2, batch_size, max_comments):
    n = x.shape[0]

    def seg_sum(vals, ids, nseg):
        out = np.zeros((nseg, vals.shape[1]), np.float32)
        np.add.at(out, ids, vals)
        return out

    def rownorm(v):
        return np.sqrt(np.maximum(np.sum(v * v, axis=1, keepdims=True), MIN_SS))

    def proj(v):
        nn = rownorm(v)
        return np.where(nn > MAXNORM, v / nn * MAXNORM, v)

    def logmap0(v):
        nn = rownorm(v)
        arg = np.minimum(nn, 1 - 1e-7)
        return v * np.arctanh(arg) / nn

    def expmap0(v):
        nn = rownorm(v)
        return v * np.tanh(nn) / nn

    h = np.tanh(seg_sum(x[src] @ W1, dst, n) + b1)
    h = np.tanh(seg_sum(h[src] @ W2, dst, n) + b2)
    h = logmap0(proj(h))
    nseg = int(batch_size) * int(max_comments)
    sums = seg_sum(h, seg_ids, nseg)
    counts = np.zeros(nseg, np.float32)
    np.add.at(counts, seg_ids, 1.0)
    agg = sums / np.maximum(counts, 1.0)[:, None]
    agg = proj(expmap0(agg))
    return agg.reshape(int(batch_size), int(max_comments), -1)


# ====================================================================
# Harness entry point: kernel(**inputs) -> np.ndarray
# ====================================================================

_CACHE = {}


def kernel(x, src, dst, seg_ids, W1, b1, W2, b2, batch_size, max_comments):
    """Full-input GNN ComEnc kernel on 8 Trainium2 NeuronCores.

    Accepts the unsharded inputs of reference.setup_inputs() and returns
    the full (batch, max_comments, HID) float32 output.
    """
    from concourse.bass_utils import run_bass_kernel_spmd

    x = np.asarray(x, dtype=np.float32)
    src = np.asarray(src).astype(np.int64)
    dst = np.asarray(dst).astype(np.int64)
    seg_ids = np.asarray(seg_ids).astype(np.int64)
    W1 = np.asarray(W1, dtype=np.float32)
    b1 = np.asarray(b1, dtype=np.float32)
    W2 = np.asarray(W2, dtype=np.float32)
    b2 = np.asarray(b2, dtype=np.float32)
    bs = int(np.asarray(batch_size))
    mc = int(np.asarray(max_comments))

    n_nodes, in_dim = x.shape
    hid = W1.shape[1]
    nseg = bs * mc
    n_cores = 8

    cfg = Cfg(n_nodes, in_dim, hid, nseg, n_cores)
    ntiles, per_core = host_prep(cfg, src, dst)

    key = (n_nodes, in_dim, hid, nseg, ntiles.tobytes())
    if key in _CACHE:
        nc = _CACHE[key]
    else:
        nc = build(cfg, ntiles)
        _CACHE.clear()
        _CACHE[key] = nc

    maps = host_inputs(cfg, x, seg_ids, W1, b1, W2, b2, per_core)
    res = run_bass_kernel_spmd(nc, maps, core_ids=list(range(n_cores)))
    partials = [r["pooled"] for r in res.results]
    out = host_epilogue(cfg, partials, bs, mc)
    return np.ascontiguousarray(out.astype(np.float32))



# revision 16
# speedup vs baseline: 1.1094x; 1.0571x over previous
"""GNN message-passing kernel for Trainium2 (8 NeuronCores, SPMD).

Computation (see reference):
  h1 = tanh(segsum(x[src] -> dst) @ W1 + b1)        [uses A(xW) = (Ax)W]
  support2 = h1 @ W2                                 (computed in L1 epilogue)
  h2 = tanh(segsum(support2[src] -> dst) + b2)
  ht = logmap0(proj(h2))  (rowwise scale)
  pooled = segment mean over seg_ids, then expmap0/proj (host epilogue)

Sharding: nodes split contiguously over cores (dst-shard). Each core owns
SHARD nodes, processes the edges whose dst is in its shard.

The spmm is a one-hot matmul over 128-edge windows.  Edges are laid out in
a per-core stream ordered (group, chunk, block); a window may span several
dst blocks, so each (window, block) pair gets its own masked one-hot
S[e,slot] = (slotcol[e]==slot) where slotcol is -1 for edges of other
blocks (DVE is_equal vs iota), accumulated into the block's PSUM acc.

Layer 1 needs no gather: the x table is a host input, so the host ships
x pre-gathered in stream order and the kernel streams it sequentially.
Layer 2 gathers support2 rows via gpsimd.dma_gather (int16 idx, tables
chunked to 32768 rows).  The only cross-core exchange is one AllGather of
support2 (bf16).
"""

import math
from contextlib import ExitStack

import numpy as np
import ml_dtypes

import concourse.bass as bass
import concourse.tile as tile
import concourse.bacc as bacc
from concourse import mybir

BF16 = mybir.dt.bfloat16
F32 = mybir.dt.float32
I16 = mybir.dt.int16
AF = mybir.ActivationFunctionType
ALU = mybir.AluOpType

MAXNORM = 1.0 - 1e-5
MIN_SS = 1e-15

SUB = 1024          # gather indices per dma_gather call (hw ring limit)
GRP = 4             # dst blocks (of 128 nodes) per PSUM group
WB = 8              # windows per L1 stream copy / L2 gather call


class Cfg:
    def __init__(self, n_nodes, in_dim, hid, n_seg, n_cores):
        self.N = n_nodes
        self.IN = in_dim
        self.HID = hid
        self.NSEG = n_seg
        self.NC = n_cores
        self.SHARD = n_nodes // n_cores
        assert self.SHARD % 128 == 0
        self.NBLK = self.SHARD // 128
        assert self.NBLK % GRP == 0
        self.NGRP = self.NBLK // GRP
        self.CH = min(32768, n_nodes)
        assert n_nodes % self.CH == 0
        self.NCHUNK = n_nodes // self.CH
        self.NSEGCH = (n_seg + 127) // 128


def _prep_layer(cfg, src, dst, nchunk):
    """Window/entry schedule for one spmm layer, SPMD-uniform across cores.

    Edges are streamed per core in (g, k, b) order; cells are (g, k) padded
    to the max count over cores, rounded up to whole 128-edge windows.

    Returns dict with:
      nw[g][k]        windows per cell
      sched[g]        list over k of list of (wglob, [[ent, b, start, stop]])
      nweff, nent     total windows / entries
      per_core        list of dicts: order (stream pos -> edge id, -1 pad),
                      slotcol [nent, 128] float32
    """
    NC = cfg.NC
    ch = cfg.N // nchunk
    core = dst // cfg.SHARD
    blk = (dst % cfg.SHARD) // 128
    slot = dst % 128
    g_all = blk // GRP
    b_all = blk % GRP
    chunk = src // ch

    cnt = np.zeros((NC, cfg.NGRP, nchunk), dtype=np.int64)
    np.add.at(cnt, (core, g_all, chunk), 1)
    nw = (cnt.max(axis=0) + 127) // 128          # [NGRP, nchunk]
    # every group needs >= 1 window (PSUM start)
    empty_g = nw.sum(axis=1) == 0
    nw[empty_g, 0] = 1
    nweff = int(nw.sum())

    cellw0 = np.zeros((cfg.NGRP, nchunk), dtype=np.int64)  # first wglob of cell
    w = 0
    for g in range(cfg.NGRP):
        for k in range(nchunk):
            cellw0[g, k] = w
            w += int(nw[g, k])

    # per-core streams
    order = np.lexsort((b_all, chunk, g_all, core))
    per_core_blk = []   # block of edge at stream pos, -1 pad
    per_core_slot = []
    per_core_ord = []
    TOT = nweff * 128
    for c in range(NC):
        sel = order[core[order] == c]
        sblk = np.full(TOT, -1, dtype=np.int64)
        sslot = np.full(TOT, -1, dtype=np.int64)
        sord = np.full(TOT, -1, dtype=np.int64)
        cg, ck = g_all[sel], chunk[sel]
        ep = 0
        for g in range(cfg.NGRP):
            for k in range(nchunk):
                n = int(cnt[c, g, k])
                pos = int(cellw0[g, k]) * 128
                if n:
                    s = sel[ep:ep + n]
                    sblk[pos:pos + n] = b_all[s]
                    sslot[pos:pos + n] = slot[s]
                    sord[pos:pos + n] = s
                    ep += n
        assert ep == len(sel)
        per_core_blk.append(sblk)
        per_core_slot.append(sslot)
        per_core_ord.append(sord)

    # entries: union over cores of (window, block) touches
    touched = np.zeros((nweff, GRP), dtype=bool)
    for c in range(NC):
        sb = per_core_blk[c].reshape(nweff, 128)
        for b in range(GRP):
            touched[:, b] |= (sb == b).any(axis=1)
    # every (g, b) needs >= 1 entry (PSUM start/stop); force in first window
    for g in range(cfg.NGRP):
        w0 = int(cellw0[g, 0])
        hi = int(cellw0[g + 1, 0]) if g + 1 < cfg.NGRP else nweff
        for b in range(GRP):
            if not touched[w0:hi, b].any():
                touched[w0, b] = True

    # entry ids in (w, b) order + schedule skeleton
    entof = np.full((nweff, GRP), -1, dtype=np.int64)
    nent = 0
    sched = []
    for g in range(cfg.NGRP):
        gs = []
        for k in range(nchunk):
            ks = []
            for lw in range(int(nw[g, k])):
                wg = int(cellw0[g, k]) + lw
                ents = []
                for b in range(GRP):
                    if touched[wg, b]:
                        entof[wg, b] = nent
                        ents.append([nent, b, False, False])
                        nent += 1
                ks.append((wg, ents))
            gs.append(ks)
        sched.append(gs)
    # start/stop flags per (g, b)
    for g in range(cfg.NGRP):
        for b in range(GRP):
            ws = [wg for k in range(nchunk) for (wg, ents) in sched[g][k]
                  if entof[wg, b] >= 0]
            first, last = ws[0], ws[-1]
            for k in range(nchunk):
                for (wg, ents) in sched[g][k]:
                    for e in ents:
                        if e[1] == b:
                            if wg == first:
                                e[2] = True
                            if wg == last:
                                e[3] = True

    # per-core slotcol tables [nent, 128]
    per_core = []
    ws_nz, bs_nz = np.nonzero(entof >= 0)
    for c in range(NC):
        sb = per_core_blk[c].reshape(nweff, 128)
        ss = per_core_slot[c].reshape(nweff, 128)
        scol = np.full((nent, 128), -1.0, dtype=np.float32)
        for wg, b in zip(ws_nz, bs_nz):
            e = entof[wg, b]
            scol[e] = np.where(sb[wg] == b, ss[wg], -1).astype(np.float32)
        per_core.append({"order": per_core_ord[c], "slotcol": scol})

    return {"nw": nw, "sched": sched, "nweff": nweff, "nent": nent,
            "cellw0": cellw0, "per_core": per_core, "nchunk": nchunk,
            "ch": ch}


def host_prep(cfg, src, dst):
    src = np.asarray(src).astype(np.int64)
    dst = np.asarray(dst).astype(np.int64)
    l1 = _prep_layer(cfg, src, dst, 1)
    l2 = _prep_layer(cfg, src, dst, cfg.NCHUNK)
    # L1 per-core src node ids (stream order) for the host-side pre-gather
    for c in range(cfg.NC):
        sord = l1["per_core"][c]["order"]
        l1["per_core"][c]["srcidx"] = np.where(
            sord >= 0, src[np.maximum(sord, 0)], 0).astype(np.int64)
    # L2 per-core idx16 tables (position within chunk, wrapped)
    for c in range(cfg.NC):
        sord = l2["per_core"][c]["order"]
        idxpos = np.where(sord >= 0, src[np.maximum(sord, 0)] % l2["ch"], 0)
        iw = idxpos.astype(np.int16).reshape(-1, 16).T      # [16, TOT/16]
        iw = np.tile(iw, (8, 1)).copy()                      # [128, TOT/16]
        l2["per_core"][c]["idx16"] = iw.astype(np.int16)
    return l1, l2


def build(cfg, l1, l2, debug_taps=False):
    """Build the Bass program. Returns nc."""
    N, IN, HID = cfg.N, cfg.IN, cfg.HID
    NW1, NW2 = l1["nweff"], l2["nweff"]
    NENT1, NENT2 = l1["nent"], l2["nent"]
    TOT2 = NW2 * 128

    nc = bacc.Bacc("TRN2", target_bir_lowering=False)

    # x pre-gathered in L1 stream order, partition-major:
    # xg[p, w*IN:(w+1)*IN] = x[src(stream pos w*128+p)]
    xg_d = nc.dram_tensor("x_gath", [128, NW1 * IN], BF16, kind="ExternalInput")
    slot1_d = nc.dram_tensor("slot1", [128, NENT1], F32, kind="ExternalInput")
    idx2_d = nc.dram_tensor("idx16", [128, TOT2 // 16], I16, kind="ExternalInput")
    slot2_d = nc.dram_tensor("slot2", [128, NENT2], F32, kind="ExternalInput")
    segid_d = nc.dram_tensor("segid", [128, cfg.NBLK], F32, kind="ExternalInput")
    iota_d = nc.dram_tensor("iota128", [128, 128], BF16, kind="ExternalInput")
    iotas_d = nc.dram_tensor("iota_seg", [128, cfg.NSEGCH * 128], F32, kind="ExternalInput")
    ident_d = nc.dram_tensor("ident", [128, 128], BF16, kind="ExternalInput")
    w1_d = nc.dram_tensor("W1", [IN, HID], BF16, kind="ExternalInput")
    w2_d = nc.dram_tensor("W2", [HID, HID], BF16, kind="ExternalInput")
    b1_d = nc.dram_tensor("b1rep", [128, HID], F32, kind="ExternalInput")
    b2_d = nc.dram_tensor("b2rep", [128, HID], F32, kind="ExternalInput")

    s2_shard = nc.dram_tensor("s2_shard", [cfg.SHARD, HID], BF16)
    s2_full = nc.dram_tensor("s2_full", [N, HID], BF16, addr_space="Shared")
    out_d = nc.dram_tensor("pooled", [cfg.NSEGCH * 128, HID + 1], F32,
                           kind="ExternalOutput")

    KIN = IN // 128   # k-chunks for W1 (2)
    mg2 = max(int(l2["nw"][g].sum()) for g in range(cfg.NGRP))
    me1 = max(sum(len(ents) for (_, ents) in l1["sched"][g][0])
              for g in range(cfg.NGRP))
    me2 = max(sum(len(ents) for k in range(l2["nchunk"])
                  for (_, ents) in l2["sched"][g][k])
              for g in range(cfg.NGRP))

    with tile.TileContext(nc) as tc, ExitStack() as ctx:
        const = ctx.enter_context(tc.tile_pool(name="const", bufs=1))
        idxp = ctx.enter_context(tc.tile_pool(name="idxp", bufs=3))
        slotp = ctx.enter_context(tc.tile_pool(name="slotp", bufs=3))
        ebufp = ctx.enter_context(tc.tile_pool(name="ebufp", bufs=3))
        sp = ctx.enter_context(tc.tile_pool(name="sp", bufs=4))
        flshp = ctx.enter_context(tc.tile_pool(name="flshp", bufs=3))
        xtp = ctx.enter_context(tc.tile_pool(name="xtp", bufs=4))
        hp = ctx.enter_context(tc.tile_pool(name="hp", bufs=3))
        h2allp = ctx.enter_context(tc.tile_pool(name="h2allp", bufs=1))
        normp = ctx.enter_context(tc.tile_pool(name="normp", bufs=1))
        htp = ctx.enter_context(tc.tile_pool(name="htp", bufs=3))

        ctx_spmm = ctx.enter_context(ExitStack())
        ps_acc = ctx_spmm.enter_context(tc.tile_pool(name="ps_acc", bufs=4, space="PSUM"))
        ps_tr = ctx_spmm.enter_context(tc.tile_pool(name="ps_tr", bufs=2, space="PSUM"))
        ps_h = ctx_spmm.enter_context(tc.tile_pool(name="ps_h", bufs=2, space="PSUM"))

        # ---- constants ----
        iota128 = const.tile([128, 128], BF16)
        nc.sync.dma_start(iota128[:], iota_d[:])
        iotaseg = const.tile([128, cfg.NSEGCH * 128], F32)
        nc.sync.dma_start(iotaseg[:], iotas_d[:])
        ident = const.tile([128, 128], BF16)
        nc.sync.dma_start(ident[:], ident_d[:])
        segid = const.tile([128, cfg.NBLK], F32)
        nc.sync.dma_start(segid[:], segid_d[:])
        w1_sb = [const.tile([128, HID], BF16, tag=f"w1_{k}", name=f"w1_{k}")
                 for k in range(KIN)]
        for k in range(KIN):
            nc.sync.dma_start(w1_sb[k][:], w1_d[k * 128:(k + 1) * 128, :])
        w2_sb = const.tile([128, HID], BF16)
        nc.sync.dma_start(w2_sb[:], w2_d[:])
        b1_sb = const.tile([128, HID], F32)
        nc.sync.dma_start(b1_sb[:], b1_d[:])
        b2_sb = const.tile([128, HID], F32)
        nc.sync.dma_start(b2_sb[:], b2_d[:])

        h2_all = h2allp.tile([128, cfg.NBLK * HID], BF16)
        norms2 = normp.tile([128, cfg.NBLK], F32)
        scale = normp.tile([128, cfg.NBLK], F32)
        na = normp.tile([128, cfg.NBLK], F32)
        nb_t = normp.tile([128, cfg.NBLK], F32)

        def l1_block(g, b, agg_ps):
            nb = g * GRP + b
            # copy PSUM f32 -> SBUF bf16
            ax = flshp.tile([128, IN], BF16, tag="ax1")
            nc.scalar.activation(ax[:], agg_ps, AF.Copy)
            h_ps = ps_h.tile([128, HID], F32, tag="hps", name="h_ps")
            for h in range(KIN):
                t_ps = ps_tr.tile([128, 128], BF16, tag="tps")
                nc.tensor.transpose(t_ps[:], ax[:, h * 128:(h + 1) * 128], ident[:])
                xt = xtp.tile([128, 128], BF16, tag="xt")
                nc.scalar.activation(xt[:], t_ps[:], AF.Copy)
                nc.tensor.matmul(h_ps[:], xt[:], w1_sb[h][:],
                                 start=(h == 0), stop=(h == KIN - 1))
            htmp = hp.tile([128, HID], F32, tag="htmp")
            nc.vector.tensor_add(htmp[:], h_ps[:], b1_sb[:])
            h1b = hp.tile([128, HID], BF16, tag="h1b")
            nc.scalar.activation(h1b[:], htmp[:], AF.Tanh)
            # support2 = h1 @ W2  (transpose h1, then W2 as moving operand)
            t2_ps = ps_tr.tile([128, 128], BF16, tag="tps")
            nc.tensor.transpose(t2_ps[:], h1b[:], ident[:])
            h1t = xtp.tile([128, 128], BF16, tag="xt")
            nc.scalar.activation(h1t[:], t2_ps[:], AF.Copy)
            s2_ps = ps_h.tile([128, HID], F32, tag="hps", name="h_ps")
            nc.tensor.matmul(s2_ps[:], h1t[:], w2_sb[:], start=True, stop=True)
            s2b = hp.tile([128, HID], BF16, tag="s2b")
            nc.scalar.activation(s2b[:], s2_ps[:], AF.Copy)
            nc.sync.dma_start(s2_shard[nb * 128:(nb + 1) * 128, :], s2b[:])

        def l2_block(g, b, agg_ps):
            nb = g * GRP + b
            htmp = hp.tile([128, HID], F32, tag="htmp")
            nc.vector.tensor_add(htmp[:], agg_ps, b2_sb[:])
            nc.scalar.activation(h2_all[:, nb * HID:(nb + 1) * HID], htmp[:],
                                 AF.Tanh)

        # ---------------- layer 1: stream pre-gathered x ----------------
        for g in range(cfg.NGRP):
            accs = [ps_acc.tile([128, IN], F32, tag="acc", name=f"acc{b}")
                    for b in range(GRP)]
            wins = l1["sched"][g][0]
            gw0 = wins[0][0]
            gnw = len(wins)
            ge0 = None
            gne = 0
            for (_, ents) in wins:
                for e in ents:
                    if ge0 is None:
                        ge0 = e[0]
                    gne += 1
            st_g = slotp.tile([128, me1], F32, tag="st1")
            nc.sync.dma_start(st_g[:, :gne], slot1_d[:, ge0:ge0 + gne])
            for w0 in range(0, gnw, WB):
                nwb = min(WB, gnw - w0)
                eb = ebufp.tile([128, WB * IN], BF16, tag="eb1")
                nc.sync.dma_start(
                    eb[:, :nwb * IN],
                    xg_d[:, (gw0 + w0) * IN:(gw0 + w0 + nwb) * IN])
                for (wg, ents) in wins[w0:w0 + nwb]:
                    j = wg - gw0 - w0
                    for (ent, b, st_f, sp_f) in ents:
                        s_t = sp.tile([128, 128], BF16, tag="s_t")
                        nc.vector.tensor_scalar(
                            s_t[:], iota128[:],
                            st_g[:, ent - ge0:ent - ge0 + 1],
                            None, ALU.is_equal)
                        nc.tensor.matmul(
                            accs[b][:, :IN], s_t[:],
                            eb[:, j * IN:(j + 1) * IN],
                            start=st_f, stop=sp_f)
            for b in range(GRP):
                l1_block(g, b, accs[b][:, :IN])

        # ---------------- exchange ----------------
        nc.gpsimd.collective_compute(
            "AllGather",
            ALU.bypass,
            ins=[s2_shard.ap().opt()],
            outs=[s2_full.ap().opt()],
            replica_groups=[list(range(cfg.NC))],
        )

        # ---------------- layer 2: gather support2 ----------------
        for g in range(cfg.NGRP):
            accs = [ps_acc.tile([128, IN], F32, tag="acc", name=f"acc{b}")
                    for b in range(GRP)]
            gw0 = int(l2["cellw0"][g, 0])
            gnw = int(l2["nw"][g].sum())
            ge0 = None
            gne = 0
            for k in range(l2["nchunk"]):
                for (_, ents) in l2["sched"][g][k]:
                    for e in ents:
                        if ge0 is None:
                            ge0 = e[0]
                        gne += 1
            it_g = idxp.tile([128, mg2 * 8], I16, tag="it2")
            nc.sync.dma_start(
                it_g[:, :gnw * 8],
                idx2_d[:, gw0 * 8:(gw0 + gnw) * 8])
            st_g = slotp.tile([128, me2], F32, tag="st2")
            nc.sync.dma_start(st_g[:, :gne], slot2_d[:, ge0:ge0 + gne])
            for k in range(l2["nchunk"]):
                wins = l2["sched"][g][k]
                if not wins:
                    continue
                tbl = s2_full[k * l2["ch"]:(k + 1) * l2["ch"], :]
                for s0 in range(0, len(wins), WB):
                    swins = wins[s0:s0 + WB]
                    nidx = len(swins) * 128
                    lw0 = swins[0][0] - gw0   # window idx within group
                    eb = ebufp.tile([128, WB * HID], BF16, tag="eb2")
                    nc.gpsimd.dma_gather(
                        out_ap=eb[:, :len(swins) * HID].rearrange(
                            "p (n f) -> p n f", f=HID),
                        in_ap=tbl,
                        idxs_ap=it_g[:, lw0 * 8:lw0 * 8 + nidx // 16],
                        num_idxs=nidx,
                        num_idxs_reg=nidx,
                        elem_size=HID,
                    )
                    for (wg, ents) in swins:
                        j = wg - swins[0][0]
                        for (ent, b, st_f, sp_f) in ents:
                            s_t = sp.tile([128, 128], BF16, tag="s_t")
                            nc.vector.tensor_scalar(
                                s_t[:], iota128[:],
                                st_g[:, ent - ge0:ent - ge0 + 1],
                                None, ALU.is_equal)
                            nc.tensor.matmul(
                                accs[b][:, :HID], s_t[:],
                                eb[:, j * HID:(j + 1) * HID],
                                start=st_f, stop=sp_f)
            for b in range(GRP):
                l2_block(g, b, accs[b][:, :HID])

        # ---------------- norms + logmap scale ----------------
        for nbk in range(cfg.NBLK):
            h2b = h2_all[:, nbk * HID:(nbk + 1) * HID]
            sq = htp.tile([128, HID], F32, tag="sq")
            nc.vector.tensor_mul(sq[:], h2b, h2b)
            nc.vector.tensor_reduce(norms2[:, nbk:nbk + 1], sq[:],
                                    mybir.AxisListType.X, ALU.add)
        # norm = sqrt(max(ss, MIN_SS)); nclip = min(norm, MAXNORM)
        nc.vector.tensor_scalar_max(na[:], norms2[:], MIN_SS)
        nc.scalar.activation(nb_t[:], na[:], AF.Sqrt)        # nb_t = norm
        nc.vector.tensor_scalar_min(na[:], nb_t[:], MAXNORM)  # na = nclip
        # artanh(nclip) = 0.5*ln((1+n)/(1-n)); scale = artanh/norm
        one_m = normp.tile([128, cfg.NBLK], F32)
        nc.vector.tensor_scalar(one_m[:], na[:], -1.0, 1.0, ALU.mult, ALU.add)
        one_p = normp.tile([128, cfg.NBLK], F32)
        nc.vector.tensor_scalar_add(one_p[:], na[:], 1.0)
        rcp = normp.tile([128, cfg.NBLK], F32)
        nc.vector.reciprocal(rcp[:], one_m[:])
        rat = normp.tile([128, cfg.NBLK], F32)
        nc.vector.tensor_mul(rat[:], one_p[:], rcp[:])
        lg = normp.tile([128, cfg.NBLK], F32)
        nc.scalar.activation(lg[:], rat[:], AF.Ln)
        nc.vector.tensor_scalar_mul(lg[:], lg[:], 0.5)
        rcpn = normp.tile([128, cfg.NBLK], F32)
        nc.vector.reciprocal(rcpn[:], nb_t[:])
        nc.vector.tensor_mul(scale[:], lg[:], rcpn[:])

        # ---------------- pooling ----------------
        ctx_spmm.close()
        ps_pool = ctx.enter_context(
            tc.tile_pool(name="ps_pool", bufs=max(cfg.NSEGCH, 1), space="PSUM"))
        pool_ps = [ps_pool.tile([128, HID + 1], F32, tag="pool", name=f"pool{sc}")
                   for sc in range(cfg.NSEGCH)]
        for nbk in range(cfg.NBLK):
            h2b = h2_all[:, nbk * HID:(nbk + 1) * HID]
            ht = htp.tile([128, HID + 1], BF16, tag="ht")
            nc.vector.tensor_scalar(ht[:, :HID], h2b, scale[:, nbk:nbk + 1],
                                    None, ALU.mult)
            nc.vector.memset(ht[:, HID:HID + 1], 1.0)
            for sc in range(cfg.NSEGCH):
                sg = sp.tile([128, 128], BF16, tag="sg")
                nc.vector.tensor_scalar(
                    sg[:], iotaseg[:, sc * 128:(sc + 1) * 128],
                    segid[:, nbk:nbk + 1], None, ALU.is_equal)
                nc.tensor.matmul(
                    pool_ps[sc][:], sg[:], ht[:],
                    start=(nbk == 0), stop=(nbk == cfg.NBLK - 1))
        for sc in range(cfg.NSEGCH):
            po = htp.tile([128, HID + 1], F32, tag="po")
            nc.vector.tensor_copy(po[:], pool_ps[sc][:])
            nc.sync.dma_start(out_d[sc * 128:(sc + 1) * 128, :], po[:])

    nc.compile()
    return nc


def host_inputs(cfg, x, seg_ids, W1, b1, W2, b2, l1, l2):
    """Per-core in_maps for run_bass_kernel_spmd."""
    N, IN, HID = cfg.N, cfg.IN, cfg.HID
    x_bf16 = np.ascontiguousarray(np.asarray(x, np.float32).astype(ml_dtypes.bfloat16))
    iota128 = np.tile(np.arange(128, dtype=np.float32), (128, 1)).astype(ml_dtypes.bfloat16)
    iotaseg = np.tile(np.arange(cfg.NSEGCH * 128, dtype=np.float32), (128, 1))
    ident = np.eye(128, dtype=np.float32).astype(ml_dtypes.bfloat16)
    w1 = np.ascontiguousarray(np.asarray(W1, np.float32).astype(ml_dtypes.bfloat16))
    w2 = np.ascontiguousarray(np.asarray(W2, np.float32).astype(ml_dtypes.bfloat16))
    b1r = np.tile(np.asarray(b1, np.float32), (128, 1))
    b2r = np.tile(np.asarray(b2, np.float32), (128, 1))
    seg = np.asarray(seg_ids, np.float32)
    NW1 = l1["nweff"]
    maps = []
    for c in range(cfg.NC):
        segc = seg[c * cfg.SHARD:(c + 1) * cfg.SHARD].reshape(cfg.NBLK, 128).T
        # pre-gathered x, partition-major [128, NW1*IN]
        rows = x_bf16[l1["per_core"][c]["srcidx"]]  # pad -> row 0 (masked)
        xg = np.ascontiguousarray(
            rows.reshape(NW1, 128, IN).transpose(1, 0, 2).reshape(128, NW1 * IN))
        maps.append({
            "x_gath": xg,
            "slot1": np.ascontiguousarray(l1["per_core"][c]["slotcol"].T),
            "idx16": l2["per_core"][c]["idx16"],
            "slot2": np.ascontiguousarray(l2["per_core"][c]["slotcol"].T),
            "segid": np.ascontiguousarray(segc),
            "iota128": iota128,
            "iota_seg": np.ascontiguousarray(iotaseg.astype(np.float32)),
            "ident": ident,
            "W1": w1,
            "W2": w2,
            "b1rep": b1r,
            "b2rep": b2r,
        })
    return maps


def host_epilogue(cfg, partials, batch_size, max_comments):
    """partials: list of per-core [NSEGCH*128, HID+1] f32."""
    acc = np.zeros_like(partials[0], dtype=np.float64)
    for p in partials:
        acc += p.astype(np.float64)
    acc = acc.astype(np.float32)
    nseg = cfg.NSEG
    sums = acc[:nseg, :cfg.HID]
    counts = acc[:nseg, cfg.HID]
    agg = sums / np.maximum(counts, 1.0)[:, None]
    # expmap0 then proj
    ss = np.maximum(np.sum(agg * agg, axis=1), MIN_SS).astype(np.float32)
    norm = np.sqrt(ss)
    y = agg * (np.tanh(norm) / norm)[:, None]
    ssy = np.maximum(np.sum(y * y, axis=1), MIN_SS).astype(np.float32)
    ny = np.sqrt(ssy)
    f = np.where(ny > MAXNORM, MAXNORM / ny, 1.0).astype(np.float32)
    y = y * f[:, None]
    return y.reshape(int(batch_size), int(max_comments), cfg.HID)


# ====================================================================
# Harness entry point: kernel(**inputs) -> np.ndarray
# ====================================================================

_CACHE = {}


def kernel(x, src, dst, seg_ids, W1, b1, W2, b2, batch_size, max_comments):
    """Full-input GNN ComEnc kernel on 8 Trainium2 NeuronCores.

    Accepts the unsharded inputs of reference.setup_inputs() and returns
    the full (batch, max_comments, HID) float32 output.
    """
    from concourse.bass_utils import run_bass_kernel_spmd

    x = np.asarray(x, dtype=np.float32)
    src = np.asarray(src).astype(np.int64)
    dst = np.asarray(dst).astype(np.int64)
    seg_ids = np.asarray(seg_ids).astype(np.int64)
    W1 = np.asarray(W1, dtype=np.float32)
    b1 = np.asarray(b1, dtype=np.float32)
    W2 = np.asarray(W2, dtype=np.float32)
    b2 = np.asarray(b2, dtype=np.float32)
    bs = int(np.asarray(batch_size))
    mc = int(np.asarray(max_comments))

    n_nodes, in_dim = x.shape
    hid = W1.shape[1]
    nseg = bs * mc
    n_cores = 8

    cfg = Cfg(n_nodes, in_dim, hid, nseg, n_cores)
    l1, l2 = host_prep(cfg, src, dst)

    key = (n_nodes, in_dim, hid, nseg, l1["nweff"], l1["nent"],
           l2["nweff"], l2["nent"])
    if key in _CACHE:
        nc = _CACHE[key]
    else:
        nc = build(cfg, l1, l2)
        _CACHE.clear()
        _CACHE[key] = nc

    maps = host_inputs(cfg, x, seg_ids, W1, b1, W2, b2, l1, l2)
    res = run_bass_kernel_spmd(nc, maps, core_ids=list(range(n_cores)))
    partials = [r["pooled"] for r in res.results]
    out = host_epilogue(cfg, partials, bs, mc)
    return np.ascontiguousarray(out.astype(np.float32))
